# revision 6
# baseline (speedup 1.0000x reference)
"""Depthwise causal conv1d (W=8) with 3 interleaved weight sets, on 8 TRN2 cores.

Reference computes r/o/a = depthwise_causal_conv(x, {rtg,obs,act}_{w,b}) and
interleaves out[:, t] = {r,o,a}[:, t] by t % 3.  Only the t%3-matching third of
each conv is needed, so total work is exactly one conv: for each output t,
out[b,t,h] = sum_k x[b, t-7+k, h] * w_{t%3}[h, k] + b_{t%3}[h].

v2 strategy (pure batch data-parallel, B=16 -> 2 per core), hybrid rows:
  - host pre-transposes x to channels-major fp16 and PHASE-SPLITS time by t%3
    (left-padded 3 zeros).  Both local batches of a channel group live in ONE
    sbuf tile [128, 2, 3082] (per-batch span padded to 3082 = even, so batch 1
    keeps 4B alignment) -> every DVE op processes both batches in one
    instruction (FD=2048, halves the 58-cycle/op overhead).
  - work unit = a (channel-group, phase) PAIR covering both batches.  Exactly
    4 of the 8 taps land at even (4B-aligned) element offsets; the other 4 are
    odd.  Engine split per pair:
      PE   the 4 odd-offset taps (+1 even tap on heavier pairs): accumulating
           fp16 diag matmuls into a [128, 2, 1024] psum tile (4 banks),
           k-outer so the 4 (b, half) matmuls of a tap share one LDWEIGHTS
           (redundant loads deduped post-compile).  Matmul rhs has no
           alignment constraint -> PE absorbs the odd offsets for free.
      ACT  one op per pair: PSUM->SBUF eviction with the bias folded in
           (fp32->fp16).  ACT is 1x-rate only, and any extra ACT product
           would cost a DVE merge add as large as the tap itself, so one
           leaf per pair is its structural maximum.
      DVE  the remaining 3-4 even taps as a fused scalar_tensor_tensor chain
           (acc' = x*w + acc, 2x mode needs the 4B alignment) seeded by the
           evicted psum partial -> no separate merge, no tree adds.
  - host re-interleaves phases / transposes back / upcasts to f32.
fp16 end-to-end rel err ~8e-4 (x, w quantization + fp16 output rounding).
"""

import os
import numpy as np

B, T, H, W = 16, 3072, 768, 8
NCORES = 8
B_LOC = B // NCORES          # 2 batches per core
G = H // 128                 # 6 channel groups
U = T // 3                   # 1024 per phase
PAD = 3                      # left zero-pad per phase (covers q in {-3..0})
UP = U + PAD                 # 1027 stored per phase
SP = 3 * UP + 1              # per-batch span in the x tile (padded to even)
NFREE = 512                  # psum window (one fp32 bank)
NPAIRS = G * 3

# how many taps the PE takes per (g, s) pair: the 4 odd-offset taps always;
# heavier pairs also get one even tap.  13x k=5 + 5x k=4 balances
# PE (~82us) against DVE (~80us) at measured per-op rates.
KPE = [4 if (g * 3 + s) % 4 == 1 else 5 for g in range(G) for s in range(3)]

_cache = {}


def _tap_off(s, k):
    """Element offset (within one batch span) of tap (s, k)'s u-window."""
    o = s + k - (W - 1)
    return (o % 3) * UP + PAD + o // 3


def _pair_taps(g, s):
    """(pe_taps, dve_taps) as lists of (k, c0) for pair (g, s)."""
    taps = [(k, _tap_off(s, k)) for k in range(W)]
    odd = [t for t in taps if t[1] % 2 == 1]
    even = [t for t in taps if t[1] % 2 == 0]
    kpe = KPE[g * 3 + s]
    pe = odd + even[: kpe - 4]
    dve = even[kpe - 4:]
    return sorted(pe), dve


def _dedupe_ldweights(nc):
    """bacc lowers every 16-bit matmul to an InstLdweights + InstMatmult pair.
    The PE serializes each load (~130ns) before its matmul.  Our loop order
    makes the 4 (b, half) matmuls of a tap share the same diag lhsT, so drop
    the redundant reloads: remove an InstLdweights whose weights AP equals the
    previous one on the PE stream, carrying its semaphore waits onto the next
    PE instruction (reverting bacc's move_matmul_waits_to_ldweights motion).
    The 64B ISA word has one wait slot, so only dedupe when the waits fit."""
    import concourse.mybir as mybir

    removed = 0
    for fn in nc.m.functions:
        for blk in fn.blocks:
            insts = list(blk.instructions)
            drop = set()
            last_key = None
            for i, inst in enumerate(insts):
                if getattr(inst, "engine", None) != mybir.EngineType.PE:
                    continue
                tn = type(inst).__name__
                if tn == "InstLdweights":
                    a = inst.ins[0]
                    key = (a.memref, a.offset, str(a.ap), str(a.dtype))
                    si = inst.sync_info
                    my_waits = list(si.on_wait) if si is not None else []
                    has_upd = si is not None and len(si.on_update) > 0
                    if key == last_key and not has_upd:
                        nxt = None
                        for j in range(i + 1, len(insts)):
                            if getattr(insts[j], "engine", None) == mybir.EngineType.PE:
                                nxt = insts[j]
                                break
                        if nxt is not None:
                            nsi = nxt.sync_info
                            n_waits = len(nsi.on_wait) if nsi is not None else 0
                            if n_waits + len(my_waits) <= 1:
                                if my_waits:
                                    if nsi is None:
                                        nxt.sync_info = mybir.SyncInfo(
                                            on_wait=my_waits, on_update=[]
                                        )
                                    else:
                                        nsi.on_wait = list(nsi.on_wait) + my_waits
                                drop.add(i)
                                removed += 1
                                continue
                    last_key = key
                elif tn == "InstMatmult":
                    pass  # non-self-loading; PE array state unchanged
                else:
                    last_key = None  # be conservative about other PE ops
            if drop:
                blk.instructions = [x for i, x in enumerate(insts) if i not in drop]
    return removed


def _build_nc():
    import concourse.bacc as bacc
    import concourse.mybir as mybir
    import concourse.tile as tile

    nc = bacc.Bacc("TRN2", target_bir_lowering=False, debug=False)
    f32 = mybir.dt.float32
    f16 = mybir.dt.float16

    x_d = nc.dram_tensor("x", [G, 128, B_LOC * SP], f16, kind="ExternalInput").ap()
    wd_d = nc.dram_tensor("wd", [NPAIRS, 128, 6 * 128], f16, kind="ExternalInput").ap()
    w_d = nc.dram_tensor("w", [128, G * 3 * W], f32, kind="ExternalInput").ap()
    b_d = nc.dram_tensor("b", [128, G * 3], f32, kind="ExternalInput").ap()
    y_d = nc.dram_tensor("y", [G, 128, B_LOC * 3 * U], f16, kind="ExternalOutput").ap()

    with tile.TileContext(nc) as tc:
        with (
            tc.tile_pool(name="const", bufs=1) as constp,
            tc.tile_pool(name="diag", bufs=2) as diagp,
            tc.tile_pool(name="xp", bufs=3) as xp,
            tc.tile_pool(name="op", bufs=2) as op,
            tc.tile_pool(name="tp", bufs=2) as tp,
            tc.tile_pool(name="dv", bufs=2) as dv,
            tc.tile_pool(name="ps", bufs=2, space="PSUM") as psp,
        ):
            wt = constp.tile([128, G * 3 * W], f32)
            bt = constp.tile([128, G * 3], f32)
            nc.sync.dma_start(wt[:], w_d[:])
            nc.sync.dma_start(bt[:], b_d[:])

            def build_diags(g, s):
                """DMA the host-built diagonal fp16 weight matrices for the
                PE taps of pair (g, s) (one contiguous transfer)."""
                pe_taps, _ = _pair_taps(g, s)
                dt_ = diagp.tile([128, len(pe_taps) * 128], f16, tag="wd")
                nc.sync.dma_start(dt_[:], wd_d[g * 3 + s][:, : len(pe_taps) * 128])
                return {
                    k: dt_[:, i * 128 : (i + 1) * 128]
                    for i, (k, _) in enumerate(pe_taps)
                }

            def load_x(g):
                xt = xp.tile([128, B_LOC, SP], f16, tag="xt")
                nc.sync.dma_start(xt[:], x_d[g])
                return xt

            xt = load_x(0)
            diags = build_diags(0, 0)
            for g in range(G):
                next_xt = load_x(g + 1) if g + 1 < G else None
                ot = op.tile([128, B_LOC, 3, U], f16, tag="ot")
                for s in range(3):
                    pe_taps, dve_taps = _pair_taps(g, s)
                    ni, nj = (g, s + 1) if s < 2 else (g + 1, 0)
                    next_diags = build_diags(ni, nj) if ni < G else None

                    ps = psp.tile([128, B_LOC, U], f32, tag="ps")
                    # k outer: the 4 (b, half) matmuls of one tap share lhsT,
                    # so the deduper elides the repeated weight loads
                    for i, (k, c0) in enumerate(pe_taps):
                        for b in range(B_LOC):
                            for nt in range(2):
                                rhs = xt[:, b, c0 + nt * NFREE : c0 + (nt + 1) * NFREE]
                                nc.tensor.matmul(
                                    ps[:, b, nt * NFREE : (nt + 1) * NFREE],
                                    diags[k], rhs,
                                    start=(i == 0), stop=(i == len(pe_taps) - 1),
                                )

                    # ACT: evict psum partial with the bias folded (its one
                    # structural op per pair) -> chain head for the DVE
                    tmpP = tp.tile([128, B_LOC, U], f16, tag="tp")
                    nc.scalar.activation(
                        tmpP[:], ps[:], mybir.ActivationFunctionType.Identity,
                        bias=bt[:, g * 3 + s : g * 3 + s + 1], scale=1.0,
                    )

                    # DVE: fused mul-add chain over the even (aligned) taps.
                    # Flat 2D per-batch APs only: multi-free-dim APs knock the
                    # op down from 2x to 1x mode (measured 2346ns vs ~690ns).
                    for b in range(B_LOC):
                        acc_ap = tmpP[:, b, :]
                        for i, (k, c0) in enumerate(dve_taps):
                            c = (g * 3 + s) * W + k
                            if i == len(dve_taps) - 1:
                                dst_ap = ot[:, b, s, :]
                            else:
                                dvt = dv.tile(
                                    [128, U], f16, tag=f"dv{b}_{i % 2}",
                                    name=f"dv{b}_{i % 2}",
                                )
                                dst_ap = dvt[:]
                            nc.vector.scalar_tensor_tensor(
                                dst_ap,
                                xt[:, b, c0 : c0 + U],
                                wt[:, c : c + 1],
                                acc_ap,
                                mybir.AluOpType.mult,
                                mybir.AluOpType.add,
                            )
                            acc_ap = dst_ap

                    diags = next_diags
                nc.sync.dma_start(y_d[g], ot[:])
                if next_xt is not None:
                    xt = next_xt

    nc.compile()
    if not os.environ.get("KERNEL_NO_LDW_DEDUP"):
        n = _dedupe_ldweights(nc)
        if os.environ.get("KERNEL_VERBOSE"):
            print(f"deduped {n} ldweights")
    return nc


def _get_nc():
    if "nc" not in _cache:
        _cache["nc"] = _build_nc()
    return _cache["nc"]


def _install_ntff_hook():
    """antenv.axon_hooks is not shipped in this container; shim it so
    bass_utils can find the NTFF profile hook (trace=True path)."""
    import sys, types
    if "antenv.axon_hooks" in sys.modules:
        return
    mod = types.ModuleType("antenv.axon_hooks")
    mod._hook = None
    mod.set_axon_ntff_profile_hook = lambda h: setattr(mod, "_hook", h)
    mod.get_axon_ntff_profile_hook = lambda: mod._hook
    sys.modules["antenv.axon_hooks"] = mod
    try:
        from trn_agent_boot.trn_boot import _ntff_profile_via_ctypes
        mod._hook = _ntff_profile_via_ctypes("/opt/axon/libaxon_pjrt.so")
    except Exception:
        mod._hook = None


def kernel(x, rtg_w, rtg_b, obs_w, obs_b, act_w, act_b):
    from concourse import bass_utils

    x = np.asarray(x, dtype=np.float32)
    w_sets = [np.asarray(a, dtype=np.float32) for a in (rtg_w, obs_w, act_w)]
    b_sets = [np.asarray(a, dtype=np.float32) for a in (rtg_b, obs_b, act_b)]

    # weights laid out [128 c_local, (g*3+s)*8+k] as f32 values (per-partition
    # scalar operands for the DVE chain)
    w_all = np.zeros((128, G * 3 * W), dtype=np.float32)
    b_all = np.zeros((128, G * 3), dtype=np.float32)
    for g in range(G):
        for s in range(3):
            w_all[:, (g * 3 + s) * W : (g * 3 + s + 1) * W] = w_sets[s][g * 128 : (g + 1) * 128]
            b_all[:, g * 3 + s] = b_sets[s][g * 128 : (g + 1) * 128]
    # host-built diagonal matmul weights, only for each pair's PE taps:
    # wd[g*3+s, ci, i*128+co] = w_s[g*128+ci, pe_taps[i]] iff ci == co
    wd = np.zeros((NPAIRS, 128, 6 * 128), dtype=np.float16)
    idx = np.arange(128)
    for g in range(G):
        for s in range(3):
            pe_taps, _ = _pair_taps(g, s)
            for i, (k, _) in enumerate(pe_taps):
                wd[g * 3 + s, idx, i * 128 + idx] = w_sets[s][g * 128 + idx, k]

    in_maps = []
    for c in range(NCORES):
        xc = x[c * B_LOC : (c + 1) * B_LOC]                      # [2, T, H]
        # xph[g, ci, b, p*UP+PAD+u] = x[b, 3u+p, g*128+ci]
        xph = np.zeros((G, 128, B_LOC, SP), dtype=np.float16)
        xv = xc.transpose(2, 0, 1).reshape(G, 128, B_LOC, U, 3)
        xph[..., : 3 * UP].reshape(G, 128, B_LOC, 3, UP)[..., PAD:] = (
            xv.transpose(0, 1, 2, 4, 3)
        )
        in_maps.append({"x": xph.reshape(G, 128, B_LOC * SP),
                        "wd": wd, "w": w_all, "b": b_all})

    nc = _get_nc()
    trace = bool(int(os.environ.get("KERNEL_TRACE", "0")))
    if trace:
        _install_ntff_hook()
    res = bass_utils.run_bass_kernel_spmd(
        nc, in_maps, core_ids=list(range(NCORES)), trace=trace,
    )
    _cache["last_result"] = res

    out = np.empty((B, T, H), dtype=np.float32)
    for c in range(NCORES):
        y = res.results[c]["y"].astype(np.float32)               # [G,128,2*3*U]
        y = y.reshape(G, 128, B_LOC, 3, U)
        # out[b, 3u+s, g*128+ci] = y[g, ci, b, s, u]
        y = y.transpose(2, 4, 3, 0, 1).reshape(B_LOC, T, H)
        out[c * B_LOC : (c + 1) * B_LOC] = y
    return out


# revision 8
# speedup vs baseline: 1.6285x; 1.6285x over previous
"""Depthwise causal conv1d (W=8) with 3 interleaved weight sets, on 8 TRN2 cores.

Reference computes r/o/a = depthwise_causal_conv(x, {rtg,obs,act}_{w,b}) and
interleaves out[:, t] = {r,o,a}[:, t] by t % 3.  Only the t%3-matching third of
each conv is needed, so total work is exactly one conv: for each output t,
out[b,t,h] = sum_k x[b, t-7+k, h] * w_{t%3}[h, k] + b_{t%3}[h].

v3 strategy (pure batch data-parallel, B=16 -> 2 per core):
  - host pre-transposes x to channels-major fp16 and PHASE-SPLITS time by t%3
    (left-padded 3 zeros).  Both local batches of a channel group share ONE
    sbuf tile [128, 2, 3082] (per-batch span padded even), so every DVE/ACT
    op covers both batches with a 3D AP in a single instruction: measured
    TENSOR_SCALAR 744ns and TENSOR_TENSOR 1216ns per [128, 2x1024] op (the
    4x / 2x perf modes survive multi-free-dim APs), vs 960/1364 per-batch.
    scalar_tensor_tensor is NOT used: its uop only exists at 1x (1280ns
    flat, 2346ns paired - measured), so fused mul-add loses to mul + add.
  - work unit = a (channel-group, phase) PAIR covering both batches; 18 per
    core, split 11 PE-pairs / 7 vec-pairs (engine loads ~80us each):
      PE pair: 8 taps as accumulating fp16 diag matmuls (32x [128,512],
           ~226ns warm), k-outer so the 4 (b, half) matmuls of a tap share
           one LDWEIGHTS (redundant loads deduped post-compile).  Diags are
           host-built and DMA'd.  ACT evicts psum -> ot directly, both
           batches in one op, bias folded (ACT is 1x-rate, so one wide op
           per pair is all it should do).
      vec pair: 8 products = m on ACT (m=4, one more on the pair that
           balances the tail; bias rides the first ACT mul) + (8-m) on DVE
           (4x tensor_scalar, any offset), then a 7-add in-place tree of
           paired tensor_tensor ops, DVE-owned products first so the tree
           overlaps the ACT muls.  ACT muls are emitted per-group ahead of
           the PE evictions so they never queue behind PE.
  - host re-interleaves phases / transposes back / upcasts to f32.
fp16 end-to-end rel err ~9e-4 (x, w quantization + fp16 output rounding).
"""

import os
import numpy as np

B, T, H, W = 16, 3072, 768, 8
NCORES = 8
B_LOC = B // NCORES          # 2 batches per core
G = H // 128                 # 6 channel groups
U = T // 3                   # 1024 per phase
PAD = 3                      # left zero-pad per phase (covers q in {-3..0})
UP = U + PAD                 # 1027 stored per phase
SP = 3 * UP + 1              # per-batch span in the x tile (padded to even)
NFREE = 512                  # psum window (one fp32 bank)
NPAIRS = G * 3

# (g*3+s) indices handled on the vector path (7 of 18, spread out so the
# DVE/ACT streams interleave with PE pairs); the rest are pure-PE pairs.
VEC_PAIRS = {1, 4, 6, 9, 11, 14, 16}
# ACT-owned products per vec pair (of 8); one pair gets 5 to trim DVE.
ACT_M = {i: (5 if i == 9 else 4) for i in VEC_PAIRS}

_cache = {}


def _tap_off(s, k):
    """Element offset (within one batch span) of tap (s, k)'s u-window."""
    o = s + k - (W - 1)
    return (o % 3) * UP + PAD + o // 3


def _dedupe_ldweights(nc):
    """bacc lowers every 16-bit matmul to an InstLdweights + InstMatmult pair.
    The PE serializes each load (~130ns) before its matmul.  Our loop order
    makes the 4 (b, half) matmuls of a tap share the same diag lhsT, so drop
    the redundant reloads: remove an InstLdweights whose weights AP equals the
    previous one on the PE stream, carrying its semaphore waits onto the next
    PE instruction (reverting bacc's move_matmul_waits_to_ldweights motion).
    The 64B ISA word has one wait slot, so only dedupe when the waits fit."""
    import concourse.mybir as mybir

    removed = 0
    for fn in nc.m.functions:
        for blk in fn.blocks:
            insts = list(blk.instructions)
            drop = set()
            last_key = None
            for i, inst in enumerate(insts):
                if getattr(inst, "engine", None) != mybir.EngineType.PE:
                    continue
                tn = type(inst).__name__
                if tn == "InstLdweights":
                    a = inst.ins[0]
                    key = (a.memref, a.offset, str(a.ap), str(a.dtype))
                    si = inst.sync_info
                    my_waits = list(si.on_wait) if si is not None else []
                    has_upd = si is not None and len(si.on_update) > 0
                    if key == last_key and not has_upd:
                        nxt = None
                        for j in range(i + 1, len(insts)):
                            if getattr(insts[j], "engine", None) == mybir.EngineType.PE:
                                nxt = insts[j]
                                break
                        if nxt is not None:
                            nsi = nxt.sync_info
                            n_waits = len(nsi.on_wait) if nsi is not None else 0
                            if n_waits + len(my_waits) <= 1:
                                if my_waits:
                                    if nsi is None:
                                        nxt.sync_info = mybir.SyncInfo(
                                            on_wait=my_waits, on_update=[]
                                        )
                                    else:
                                        nsi.on_wait = list(nsi.on_wait) + my_waits
                                drop.add(i)
                                removed += 1
                                continue
                    last_key = key
                elif tn == "InstMatmult":
                    pass  # non-self-loading; PE array state unchanged
                else:
                    last_key = None  # be conservative about other PE ops
            if drop:
                blk.instructions = [x for i, x in enumerate(insts) if i not in drop]
    return removed


def _build_nc():
    import concourse.bacc as bacc
    import concourse.mybir as mybir
    import concourse.tile as tile

    nc = bacc.Bacc("TRN2", target_bir_lowering=False, debug=False)
    f32 = mybir.dt.float32
    f16 = mybir.dt.float16
    AL = mybir.AluOpType

    x_d = nc.dram_tensor("x", [G, 128, B_LOC * SP], f16, kind="ExternalInput").ap()
    wd_d = nc.dram_tensor("wd", [NPAIRS, 128, W * 128], f16, kind="ExternalInput").ap()
    w_d = nc.dram_tensor("w", [128, G * 3 * W], f32, kind="ExternalInput").ap()
    b_d = nc.dram_tensor("b", [128, G * 3], f32, kind="ExternalInput").ap()
    y_d = nc.dram_tensor("y", [G, 128, B_LOC * 3 * U], f16, kind="ExternalOutput").ap()

    with tile.TileContext(nc) as tc:
        with (
            tc.tile_pool(name="const", bufs=1) as constp,
            tc.tile_pool(name="diag", bufs=2) as diagp,
            tc.tile_pool(name="xp", bufs=3) as xp,
            tc.tile_pool(name="op", bufs=2) as op,
            tc.tile_pool(name="pr", bufs=2) as pr,
            tc.tile_pool(name="ps", bufs=2, space="PSUM") as psp,
        ):
            wt = constp.tile([128, G * 3 * W], f32)
            bt = constp.tile([128, G * 3], f32)
            nc.sync.dma_start(wt[:], w_d[:])
            nc.sync.dma_start(bt[:], b_d[:])

            def build_diags(g, s):
                """DMA the host-built diag fp16 weight matrices for PE pair
                (g, s); returns None for vec pairs."""
                if g >= G or g * 3 + s in VEC_PAIRS:
                    return None
                dt_ = diagp.tile([128, W * 128], f16, tag="wd")
                nc.sync.dma_start(dt_[:], wd_d[g * 3 + s])
                return {k: dt_[:, k * 128 : (k + 1) * 128] for k in range(W)}

            def load_x(g):
                xt = xp.tile([128, B_LOC, SP], f16, tag="xt")
                nc.sync.dma_start(xt[:], x_d[g])
                return xt

            def wcol(g, s, k):
                c = (g * 3 + s) * W + k
                return wt[:, c : c + 1]

            xt = load_x(0)
            diags = build_diags(0, 0)
            for g in range(G):
                next_xt = load_x(g + 1) if g + 1 < G else None
                ot = op.tile([128, B_LOC, 3, U], f16, tag="ot")

                # vec-pair products first: ACT muls depend only on the x DMA,
                # so ACT never queues them behind a PE eviction; DVE products
                # follow so the in-place tree has early operands
                prods = {}
                for s in range(3):
                    if g * 3 + s not in VEC_PAIRS:
                        continue
                    m = ACT_M[g * 3 + s]
                    bias = bt[:, g * 3 + s : g * 3 + s + 1]
                    pts = []
                    for j in range(W):
                        pt = pr.tile(
                            [128, B_LOC, U], f16, tag=f"pr{j}", name=f"pr{j}"
                        )
                        pts.append(pt)
                    for j in range(m):
                        c0 = _tap_off(s, j)
                        nc.scalar.activation(
                            pts[j][:], xt[:, :, c0 : c0 + U],
                            mybir.ActivationFunctionType.Identity,
                            bias=bias if j == 0 else 0.0, scale=wcol(g, s, j),
                        )
                    prods[s] = (m, pts)
                for s, (m, pts) in prods.items():
                    for j in range(m, W):
                        c0 = _tap_off(s, j)
                        nc.vector.tensor_scalar_mul(
                            pts[j][:], xt[:, :, c0 : c0 + U], wcol(g, s, j)
                        )

                for s in range(3):
                    idx = g * 3 + s
                    ni, nj = (g, s + 1) if s < 2 else (g + 1, 0)
                    next_diags = build_diags(ni, nj)

                    if idx not in VEC_PAIRS:
                        # PE pair: 8 accumulating diag matmul taps, k outer so
                        # the deduper elides repeated weight loads
                        ps = psp.tile([128, B_LOC, U], f32, tag="ps")
                        for i in range(W):
                            c0 = _tap_off(s, i)
                            for b in range(B_LOC):
                                for nt in range(2):
                                    rhs = xt[:, b, c0 + nt * NFREE : c0 + (nt + 1) * NFREE]
                                    nc.tensor.matmul(
                                        ps[:, b, nt * NFREE : (nt + 1) * NFREE],
                                        diags[i], rhs,
                                        start=(i == 0), stop=(i == W - 1),
                                    )
                        # ACT: single paired eviction straight into the output
                        # tile with the bias folded
                        nc.scalar.activation(
                            ot[:, :, s, :], ps[:],
                            mybir.ActivationFunctionType.Identity,
                            bias=bt[:, idx : idx + 1], scale=1.0,
                        )
                    else:
                        # vec pair: 7-add in-place tree over the products,
                        # DVE-owned (j >= m) pairs first
                        m, pts = prods[s]
                        if m == 4:
                            order = [(4, 5), (6, 7), (4, 6), (0, 1), (2, 3), (0, 2)]
                            final = (0, 4)
                        else:  # m == 5
                            order = [(5, 6), (5, 7), (0, 1), (2, 3), (0, 2), (0, 4)]
                            final = (0, 5)
                        for a_, b_ in order:
                            nc.vector.tensor_add(pts[a_][:], pts[a_][:], pts[b_][:])
                        nc.vector.tensor_add(
                            ot[:, :, s, :], pts[final[0]][:], pts[final[1]][:]
                        )
                    diags = next_diags
                nc.sync.dma_start(y_d[g], ot[:])
                if next_xt is not None:
                    xt = next_xt

    nc.compile()
    if not os.environ.get("KERNEL_NO_LDW_DEDUP"):
        n = _dedupe_ldweights(nc)
        if os.environ.get("KERNEL_VERBOSE"):
            print(f"deduped {n} ldweights")
    return nc


def _get_nc():
    if "nc" not in _cache:
        _cache["nc"] = _build_nc()
    return _cache["nc"]


def _install_ntff_hook():
    """antenv.axon_hooks is not shipped in this container; shim it so
    bass_utils can find the NTFF profile hook (trace=True path)."""
    import sys, types
    if "antenv.axon_hooks" in sys.modules:
        return
    mod = types.ModuleType("antenv.axon_hooks")
    mod._hook = None
    mod.set_axon_ntff_profile_hook = lambda h: setattr(mod, "_hook", h)
    mod.get_axon_ntff_profile_hook = lambda: mod._hook
    sys.modules["antenv.axon_hooks"] = mod
    try:
        from trn_agent_boot.trn_boot import _ntff_profile_via_ctypes
        mod._hook = _ntff_profile_via_ctypes("/opt/axon/libaxon_pjrt.so")
    except Exception:
        mod._hook = None


def kernel(x, rtg_w, rtg_b, obs_w, obs_b, act_w, act_b):
    from concourse import bass_utils

    x = np.asarray(x, dtype=np.float32)
    w_sets = [np.asarray(a, dtype=np.float32) for a in (rtg_w, obs_w, act_w)]
    b_sets = [np.asarray(a, dtype=np.float32) for a in (rtg_b, obs_b, act_b)]

    # weights laid out [128 c_local, (g*3+s)*8+k] as f32 values (per-partition
    # scalar operands for the DVE/ACT muls)
    w_all = np.zeros((128, G * 3 * W), dtype=np.float32)
    b_all = np.zeros((128, G * 3), dtype=np.float32)
    for g in range(G):
        for s in range(3):
            w_all[:, (g * 3 + s) * W : (g * 3 + s + 1) * W] = w_sets[s][g * 128 : (g + 1) * 128]
            b_all[:, g * 3 + s] = b_sets[s][g * 128 : (g + 1) * 128]
    # host-built diagonal matmul weights for the PE pairs:
    # wd[g*3+s, ci, k*128+co] = w_s[g*128+ci, k] iff ci == co
    wd = np.zeros((NPAIRS, 128, W * 128), dtype=np.float16)
    idx = np.arange(128)
    for g in range(G):
        for s in range(3):
            if g * 3 + s in VEC_PAIRS:
                continue
            for k in range(W):
                wd[g * 3 + s, idx, k * 128 + idx] = w_sets[s][g * 128 + idx, k]

    in_maps = []
    for c in range(NCORES):
        xc = x[c * B_LOC : (c + 1) * B_LOC]                      # [2, T, H]
        # xph[g, ci, b, p*UP+PAD+u] = x[b, 3u+p, g*128+ci]
        xph = np.zeros((G, 128, B_LOC, SP), dtype=np.float16)
        xv = xc.transpose(2, 0, 1).reshape(G, 128, B_LOC, U, 3)
        xph[..., : 3 * UP].reshape(G, 128, B_LOC, 3, UP)[..., PAD:] = (
            xv.transpose(0, 1, 2, 4, 3)
        )
        in_maps.append({"x": xph.reshape(G, 128, B_LOC * SP),
                        "wd": wd, "w": w_all, "b": b_all})

    nc = _get_nc()
    trace = bool(int(os.environ.get("KERNEL_TRACE", "0")))
    if trace:
        _install_ntff_hook()
    res = bass_utils.run_bass_kernel_spmd(
        nc, in_maps, core_ids=list(range(NCORES)), trace=trace,
    )
    _cache["last_result"] = res

    out = np.empty((B, T, H), dtype=np.float32)
    for c in range(NCORES):
        y = res.results[c]["y"].astype(np.float32)               # [G,128,2*3*U]
        y = y.reshape(G, 128, B_LOC, 3, U)
        # out[b, 3u+s, g*128+ci] = y[g, ci, b, s, u]
        y = y.transpose(2, 4, 3, 0, 1).reshape(B_LOC, T, H)
        out[c * B_LOC : (c + 1) * B_LOC] = y
    return out
